# revision 67
# baseline (speedup 1.0000x reference)
"""HSTU block kernel for 8 trn2 NeuronCores (v3).

Sharding: core c -> batch c//2, head-group c%2 (4 of 8 heads).
Per pair (2b, 2b+1): attention head-split; LN(attn) stats via tiny pairwise
AllReduces (two halves, overlapped with attention); output projection
computed as partial sums over each core's 256 attn dims and pairwise
AllReduce-added straight into the output tensor.

The rel-bias staircase is expanded on device by a prefix-sum scan (gpsimd)
over a host-staged impulse canvas E:
 - each 512-col chunk's start column holds the exact bias, so scan chunks
   are independent (init 0) and start at column 512*(jt//4);
 - on diagonal chunks the scan's second operand adds exact +-64 steps that
   make silu(qk + bias) vanish above the causal diagonal (no mask pass).

Engine budget per core: PE does projections/attention/output matmuls (fp16
operands, f32 psum), ACT does all silu/LN work with batched Sqrt (3 table
switches total), DVE does reductions/casts/psum evacuation, GpSimd does the
scan + collectives, DMA engines do the fp16 transposes (XBAR).
"""

import numpy as np
from contextlib import ExitStack

B, N, D = 4, 2048, 512
H, DV, DQ = 8, 64, 64
NT = N // 128          # 16 token tiles
EPS = 1e-5
MASK_OFF = 64.0        # silu(qk + bias - 64) == 0 above the causal diagonal

_CACHE = {}


# ---------------------------------------------------------------- host metadata
def _bucket_table():
    d_all = np.arange(0, 1000001, dtype=np.float32)
    buck = np.clip((np.log(np.maximum(d_all, 1.0)) / np.float32(0.301)).astype(np.int32), 0, 128)
    kmax = int(buck.max())
    T = np.searchsorted(buck, np.arange(1, kmax + 1), side="left")
    return buck, T, kmax


def _build_E(ts_b, ts_w, pos_w, buck, T, kmax):
    """Impulse canvas E [j, i]: cumsum along i from each 512-chunk start
    column reproduces bias^T (key-major)."""
    c = ts_b.astype(np.int64)
    r = np.concatenate([ts_b[1:], ts_b[-1:]]).astype(np.int64)
    tw = ts_w.astype(np.float32)
    delta = tw[1:kmax + 1] - tw[0:kmax]
    E = np.zeros((N, N), dtype=np.float32)
    Dp = (pos_w[:-1] - pos_w[1:]).astype(np.float32)
    jj = np.arange(N)
    ii = np.arange(1, N)
    # Toeplitz pos deltas: E[j, i>=1] += Dp[N-1+j-i]
    E[:, 1:] += Dp[(N - 1 + jj[:, None] - ii[None, :])]
    for k in range(kmax):
        lo = np.searchsorted(r, c - T[k], side="right")
        hi = np.searchsorted(r, c + T[k], side="left")
        valid = lo < hi
        l2, h2, jv = lo[valid], hi[valid], jj[valid]
        m = (l2 >= 1) & (l2 < N)
        np.add.at(E, (jv[m], l2[m]), -delta[k])
        m = (h2 >= 1) & (h2 < N)
        np.add.at(E, (jv[m], h2[m]), delta[k])
    # exact bias at every 512-chunk start column >= the row's group start
    for ci in range(4):
        s = 512 * ci
        rows = jj[(jj // 512) <= ci]
        d0 = np.abs(r[s] - c[rows])
        base = tw[buck[d0]] + pos_w[N - 1 + rows - s].astype(np.float32)
        E[rows, s] = base
    return E


def _build_M():
    """Mask-step canvas for diagonal chunks, [128, 4, 512] (indexed by
    jt%4): -MASK_OFF at local col 0 (rows below the chunk-start row),
    +MASK_OFF at the causal diagonal. Exact in fp16."""
    M = np.zeros((128, 4, 512), dtype=np.float32)
    for q in range(4):
        for p in range(128):
            dcol = 128 * q + p
            if dcol > 0:
                M[p, q, 0] -= MASK_OFF
                M[p, q, dcol] += MASK_OFF
    return M.astype(np.float16)


# ---------------------------------------------------------------- device kernel
def _build_nc(no_cc=False):
    import concourse.bass as bass
    import concourse.bacc as bacc
    import concourse.mybir as mybir
    import concourse.tile as tile

    f32 = mybir.dt.float32
    f16 = mybir.dt.float16
    AF = mybir.ActivationFunctionType
    ALU = mybir.AluOpType
    AX = mybir.AxisListType

    nc = bacc.Bacc(num_devices=8)

    x_in = nc.dram_tensor("x", [N, D], f32, kind="ExternalInput")
    wuv_in = nc.dram_tensor("wuv", [D, 1024], f16, kind="ExternalInput")
    wo_in = nc.dram_tensor("wo", [256, D], f16, kind="ExternalInput")
    ob_in = nc.dram_tensor("ob", [1, D], f16, kind="ExternalInput")
    E_in = nc.dram_tensor("E", [N, N], f16, kind="ExternalInput")
    padv_in = nc.dram_tensor("padv", [128, NT], f32, kind="ExternalInput")
    padr_in = nc.dram_tensor("padr", [128, NT], f32, kind="ExternalInput")
    idb_in = nc.dram_tensor("idb", [128, 128], f16, kind="ExternalInput")
    ones_in = nc.dram_tensor("ones1", [1, 128], f16, kind="ExternalInput")
    M_in = nc.dram_tensor("M", [128, 4 * 512], f16, kind="ExternalInput")
    padxr_in = nc.dram_tensor("padxr", [128, NT], f32, kind="ExternalInput")
    sin_t = [nc.dram_tensor(f"sin{h}", [128, 16], f32) for h in range(2)]
    sout_t = [nc.dram_tensor(f"sout{h}", [128, 16], f32) for h in range(2)]
    opart_t = nc.dram_tensor("opart", [N, D], f32)
    ored_t = nc.dram_tensor("ored", [N, D], f32)
    out_t = nc.dram_tensor("out", [N, D], f32, kind="ExternalOutput")

    pairs = [[0, 1], [2, 3], [4, 5], [6, 7]]

    with tile.TileContext(nc) as tc, ExitStack() as top:
        cpool = top.enter_context(tc.tile_pool(name="consts", bufs=1))
        idb = cpool.tile([128, 128], f16)
        ones1 = cpool.tile([1, 128], f16)
        obr = cpool.tile([1, D], f16)
        padv = cpool.tile([128, NT], f32)
        padr = cpool.tile([128, NT], f32)
        padxr = cpool.tile([128, NT], f32)
        epst = cpool.tile([128, 1], f32)
        nc.vector.memset(epst[:], EPS)
        nc.sync.dma_start(padxr[:], padxr_in[:, :])
        Mt = cpool.tile([128, 4, 512], f16)
        wq = [cpool.tile([128, 1024], f16, tag=f"wq{k}", name=f"wq{k}") for k in range(4)]
        wo = [cpool.tile([128, D], f16, tag=f"wo{k}", name=f"wo{k}") for k in range(2)]
        nc.sync.dma_start(Mt[:], M_in[:, :])
        nc.sync.dma_start(idb[:], idb_in[:, :])
        nc.sync.dma_start(ones1[:], ones_in[:, :])
        nc.sync.dma_start(obr[:], ob_in[:, :])
        nc.sync.dma_start(padv[:], padv_in[:, :])
        nc.sync.dma_start(padr[:], padr_in[:, :])
        for k in range(4):
            nc.sync.dma_start(wq[k][:], wuv_in[k * 128:(k + 1) * 128, :])
        for k in range(2):
            nc.sync.dma_start(wo[k][:], wo_in[k * 128:(k + 1) * 128, :])

        # resident activation storage
        rpool = top.enter_context(tc.tile_pool(name="resid", bufs=1))
        normT = rpool.tile([128, 4, N], f16)
        qT = [rpool.tile([128, N], f16, tag=f"qT{p}", name=f"qT{p}") for p in range(2)]
        kT = [rpool.tile([128, N], f16, tag=f"kT{p}", name=f"kT{p}") for p in range(2)]
        uvt = [rpool.tile([128, 512], f16, tag=f"uv{t}", name=f"uv{t}") for t in range(NT)]
        avt = [rpool.tile([128, 256], f16, tag=f"avt{t}", name=f"avt{t}") for t in range(NT)]
        bias = [rpool.tile([128, N - 512 * (jt // 4)], f16, tag=f"bias{jt}", name=f"bias{jt}")
                for jt in range(NT)]
        oT = rpool.tile([128, 2, N], f16)
        xf = [rpool.tile([128, D], f16, tag=f"xf{t}", name=f"xf{t}") for t in range(NT)]
        s1sum = rpool.tile([128, NT], f32)
        s1sq = rpool.tile([128, NT], f32)
        stS = rpool.tile([128, NT], f32)
        stQ = rpool.tile([128, NT], f32)
        mu1 = rpool.tile([128, NT], f32)
        rs1 = rpool.tile([128, NT], f32)
        nm1 = rpool.tile([128, NT], f32)
        rs5 = rpool.tile([128, NT], f32)
        nm5 = rpool.tile([128, NT], f32)

        # long-lived working pools
        xp = top.enter_context(tc.tile_pool(name="xly", bufs=1))
        scrp = top.enter_context(tc.tile_pool(name="scr", bufs=1))
        ep = top.enter_context(tc.tile_pool(name="escan", bufs=2))
        sp = top.enter_context(tc.tile_pool(name="stat", bufs=2))
        nrmp = top.enter_context(tc.tile_pool(name="nrm", bufs=2))
        oinp = top.enter_context(tc.tile_pool(name="oin", bufs=3))
        osbp = top.enter_context(tc.tile_pool(name="osb", bufs=2))
        s5p = top.enter_context(tc.tile_pool(name="stat5", bufs=2))
        wpool = top.enter_context(tc.tile_pool(name="wprime", bufs=1))
        scr4p = top.enter_context(tc.tile_pool(name="scr4", bufs=3))
        pbig = top.enter_context(tc.tile_pool(name="pbig", bufs=2, space="PSUM"))
        pqk = top.enter_context(tc.tile_pool(name="pqk", bufs=2, space="PSUM"))
        pav = top.enter_context(tc.tile_pool(name="pav", bufs=2, space="PSUM"))
        wp = [wpool.tile([128, 2, 512], f16, tag=f"wp{jt}", name=f"wp{jt}")
              for jt in range(NT)]

        # ----- thunk helpers ------------------------------------------------
        def emit(thunks):
            for th in thunks:
                th()

        def interleave(a, b):
            """Emit a and b round-robin, proportionally."""
            na, nb = len(a), len(b)
            if nb == 0:
                emit(a)
                return
            ia = ib = 0
            while ia < na or ib < nb:
                # keep a's progress ratio ahead of b's
                if ib >= nb or (ia < na and ia * nb <= ib * na):
                    a[ia]()
                    ia += 1
                else:
                    b[ib]()
                    ib += 1

        def scan_jt(jt, chunks=None):
            # walrus only lowers TensorTensorScanArith on DVE
            eng = nc.vector
            g = jt // 4
            cl = list(chunks) if chunks is not None else list(range(g, 4))
            c0 = cl[0]
            wdt = (cl[-1] + 1 - c0) * 512
            et = ep.tile([128, N], f16, tag="E")
            nc.sync.dma_start(et[:, :wdt],
                              E_in[jt * 128:(jt + 1) * 128,
                                   512 * c0:512 * c0 + wdt])
            for c in cl:
                lo = c * 512 - 512 * g          # col inside bias[jt]
                el = c * 512 - 512 * c0         # col inside the E tile
                if c == g:
                    eng.tensor_tensor_scan(bias[jt][:, lo:lo + 512],
                                           et[:, el:el + 512],
                                           Mt[:, jt % 4, :],
                                           0.0, ALU.add, ALU.add)
                else:
                    eng.tensor_tensor_scan(bias[jt][:, lo:lo + 512],
                                           et[:, el:el + 512], et[:, el:el + 512],
                                           0.0, ALU.add, ALU.bypass)

        xtiles = {}

        def stats_thunks(grp):
            ths = []
            for t in range(grp * 8, grp * 8 + 8):
                def th(t=t):
                    xs = xp.tile([128, D], f32, tag=f"x{t % 9}")
                    xtiles[t] = xs
                    nc.sync.dma_start(xs[:], x_in[t * 128:(t + 1) * 128, :])
                    scr = scrp.tile([128, D], f16, tag="scr")
                    nc.scalar.activation(scr[:], xs[:], AF.Square,
                                         accum_out=s1sq[:, t:t + 1])
                    nc.vector.tensor_reduce(s1sum[:, t:t + 1], xs[:], AX.X, ALU.add)
                    if t < 4:
                        # earliest-needed bias chunk (attention c0) during
                        # the stats pass while DVE still has slack
                        scan_jt(t, chunks=[0])
                ths.append(th)
            return ths

        def batch1(grp):
            gs = slice(grp * 8, grp * 8 + 8)
            var1 = sp.tile([128, 8], f32, tag="v1")
            nc.vector.tensor_scalar_mul(mu1[:, gs], s1sum[:, gs], 1.0 / D)
            nc.vector.scalar_tensor_tensor(var1[:], mu1[:, gs], -1.0, mu1[:, gs],
                                           ALU.mult, ALU.mult)
            nc.vector.scalar_tensor_tensor(var1[:], s1sq[:, gs], 1.0 / D, var1[:],
                                           ALU.mult, ALU.add)
            sd1 = sp.tile([128, 8], f32, tag="sd1")
            nc.scalar.activation(sd1[:], var1[:], AF.Sqrt, bias=epst[:])
            nc.vector.reciprocal(rs1[:, gs], sd1[:])
            nc.vector.scalar_tensor_tensor(nm1[:, gs], mu1[:, gs], -1.0, rs1[:, gs],
                                           ALU.mult, ALU.mult)

        def apply_thunks(grp):
            ths = []
            for t in range(grp * 8, grp * 8 + 8):
                def th(t=t):
                    nrm = nrmp.tile([128, D], f16, tag="nrm")
                    nc.scalar.activation(nrm[:], xtiles[t][:], AF.Identity,
                                         bias=nm1[:, t:t + 1], scale=rs1[:, t:t + 1])
                    nc.sync.dma_start_transpose(
                        normT[:, :, t * 128:(t + 1) * 128], nrm[:])
                    # fp16 cast of x with (pad-row x even-core) mask pre-folded
                    nc.vector.tensor_scalar_mul(xf[t][:], xtiles[t][:],
                                                padxr[:, t:t + 1])
                    if t >= 4:
                        scan_jt(t)
                    else:
                        scan_jt(t, chunks=[1, 2, 3])
                    ps = pbig.tile([128, 512], f32, tag="big")
                    for k in range(4):
                        nc.tensor.matmul(ps[:], normT[:, k, t * 128:(t + 1) * 128],
                                         wq[k][:, 0:512], start=(k == 0), stop=(k == 3))
                    nc.scalar.activation(uvt[t][:], ps[:], AF.Silu)
                    nc.vector.tensor_scalar_mul(uvt[t][:, 256:512],
                                                uvt[t][:, 256:512], padv[:, t:t + 1])
                    if t % 4 == 3:
                        c = t // 4
                        for p in range(2):
                            for col0 in (512, 768):  # q block, k block
                                ps = pbig.tile([128, 512], f32, tag="big")
                                for k in range(4):
                                    nc.tensor.matmul(
                                        ps[:],
                                        wq[k][:, col0 + p * 128:col0 + 128 + p * 128],
                                        normT[:, k, c * 512:(c + 1) * 512],
                                        start=(k == 0), stop=(k == 3))
                                dst = qT[p] if col0 == 512 else kT[p]
                                nc.scalar.activation(
                                    dst[:, c * 512:(c + 1) * 512], ps[:], AF.Silu)
                ths.append(th)
            return ths

        # ----- attention thunks --------------------------------------------
        def attn_thunks(P, c):
            ths = []
            for jt in range(4 * c + 4):
                def th(jt=jt, c=c, P=P):
                    g = jt // 4
                    off = 512 * g
                    lo = jt * 128 if c == g else c * 512
                    wdt = (c + 1) * 512 - lo
                    wlo = lo - c * 512
                    ps = pqk.tile([128, 2, 512], f32, tag="qk")
                    for hh in range(2):
                        nc.tensor.matmul(ps[:, hh, :wdt], idb[:],
                                         bias[jt][:, lo - off:lo - off + wdt],
                                         start=True, stop=False,
                                         skip_group_check=(hh == 1))
                        nc.tensor.matmul(
                            ps[:, hh, :wdt],
                            kT[P][64 * hh:64 * (hh + 1), jt * 128:(jt + 1) * 128],
                            qT[P][64 * hh:64 * (hh + 1), lo:lo + wdt],
                            start=False, stop=True, skip_group_check=True)
                    nc.scalar.activation(wp[jt][:, :, wlo:wlo + wdt],
                                         ps[:, :, :wdt], AF.Silu)
                ths.append(th)
            for it in range(4 * c, 4 * c + 4):
                def th(it=it, c=c, P=P):
                    pa = pav.tile([128, 128], f32, tag="av")
                    il = it * 128 - c * 512
                    # hh OUTER: interleaving two start=True groups in one
                    # bank loses the first group's accumulation (start
                    # clears the bank's has_written bits)
                    for hh in range(2):
                        for jt in range(it + 1):
                            nc.tensor.matmul(
                                pa[:, 64 * hh:64 * (hh + 1)],
                                wp[jt][:, hh, il:il + 128],
                                uvt[jt][:, 256 + 64 * (2 * P + hh):
                                         320 + 64 * (2 * P + hh)],
                                start=(jt == 0), stop=(jt == it),
                                skip_group_check=(hh == 1))
                    nc.vector.tensor_copy(avt[it][:, 128 * P:128 * (P + 1)], pa[:])
                    if P == 1:
                        # both pairs landed: per-token LN stats over all 256
                        # local attn dims in one pass
                        sq4 = scr4p.tile([128, 256], f16, tag="s4")
                        nc.vector.tensor_tensor(sq4[:], avt[it][:], avt[it][:],
                                                ALU.mult)
                        nc.vector.tensor_reduce(stQ[:, it:it + 1], sq4[:],
                                                AX.X, ALU.add)
                        nc.vector.tensor_reduce(stS[:, it:it + 1], avt[it][:],
                                                AX.X, ALU.add)
                ths.append(th)
            return ths

        def ship_stats(h):
            its = slice(h * 8, h * 8 + 8)
            nc.sync.dma_start(sin_t[h][:, 0:8], stS[:, its])
            nc.sync.dma_start(sin_t[h][:, 8:16], stQ[:, its])
            if not no_cc:
                nc.gpsimd.collective_compute(
                    "AllReduce", mybir.AluOpType.add, replica_groups=pairs,
                    ins=[sin_t[h][:, :]], outs=[sout_t[h][:, :]])

        def stats5(h):
            src = sin_t[h] if no_cc else sout_t[h]
            ar = s5p.tile([128, 16], f32, tag="ar")
            nc.sync.dma_start(ar[:], src[:, 0:16])
            gs = slice(h * 8, h * 8 + 8)
            mu5 = s5p.tile([128, 8], f32, tag="mu5")
            nc.vector.tensor_scalar_mul(mu5[:], ar[:, 0:8], 1.0 / 512)
            vr5 = s5p.tile([128, 8], f32, tag="vr5")
            nc.vector.scalar_tensor_tensor(vr5[:], mu5[:], -1.0, mu5[:],
                                           ALU.mult, ALU.mult)
            nc.vector.scalar_tensor_tensor(vr5[:], ar[:, 8:16], 1.0 / 512, vr5[:],
                                           ALU.mult, ALU.add)
            sd5 = s5p.tile([128, 8], f32, tag="sd5")
            nc.scalar.activation(sd5[:], vr5[:], AF.Sqrt, bias=epst[:])
            nc.vector.reciprocal(rs5[:, gs], sd5[:])
            nc.vector.scalar_tensor_tensor(nm5[:, gs], mu5[:], -1.0, rs5[:, gs],
                                           ALU.mult, ALU.mult)

        def phase5_thunks(h):
            ths = []
            for t in range(h * 8, h * 8 + 8):
                def th(t=t):
                    ln = oinp.tile([128, 256], f16, tag="ln")
                    nc.scalar.activation(ln[:], avt[t][:], AF.Identity,
                                         bias=nm5[:, t:t + 1], scale=rs5[:, t:t + 1])
                    oin = oinp.tile([128, 256], f16, tag="oin")
                    nc.vector.tensor_tensor(oin[:], ln[:], uvt[t][:, 0:256], ALU.mult)
                    nc.sync.dma_start_transpose(
                        oT[:, :, t * 128:(t + 1) * 128], oin[:])
                    ps = pbig.tile([128, 512], f32, tag="big")
                    for k in range(2):
                        nc.tensor.matmul(ps[:], oT[:, k, t * 128:(t + 1) * 128],
                                         wo[k][:], start=(k == 0), stop=False)
                    nc.tensor.matmul(ps[:], ones1[:], obr[:],
                                     start=False, stop=True, skip_group_check=True)
                    osb5 = osbp.tile([128, D], f32, tag="osb5")
                    nc.vector.scalar_tensor_tensor(osb5[:], ps[:], padr[:, t:t + 1],
                                                   xf[t][:], ALU.mult, ALU.add)
                    nc.sync.dma_start(opart_t[t * 128:(t + 1) * 128, :], osb5[:])
                ths.append(th)
            return ths

        def ship_out(h):
            rows = slice(1024 * h, 1024 * (h + 1))
            if not no_cc:
                nc.gpsimd.collective_compute(
                    "AllReduce", mybir.AluOpType.add, replica_groups=pairs,
                    ins=[opart_t[rows, :]], outs=[ored_t[rows, :]])
                nc.sync.dma_start(out_t[rows, :], ored_t[rows, :])

        # ----- schedule -----------------------------------------------------
        emit(stats_thunks(0))
        batch1(0)
        a0 = apply_thunks(0)
        emit(a0[:4])
        interleave(attn_thunks(0, 0), a0[4:] + stats_thunks(1)[:4])
        interleave(attn_thunks(1, 0), stats_thunks(1)[4:])
        batch1(1)
        interleave(attn_thunks(0, 1) + attn_thunks(1, 1), apply_thunks(1))
        ship_stats(0)
        emit(attn_thunks(0, 2) + attn_thunks(1, 2))
        stats5(0)
        interleave(attn_thunks(0, 3) + attn_thunks(1, 3), phase5_thunks(0))
        ship_stats(1)
        ship_out(0)
        stats5(1)
        emit(phase5_thunks(1))
        ship_out(1)
        if no_cc:
            # sim-only fallback so out_t is produced without collectives
            nc.sync.dma_start(out_t[:, :], opart_t[:, :])

    nc.compile()
    return nc


# ---------------------------------------------------------------- entry point
def kernel(**inputs):
    x = np.asarray(inputs["x"], dtype=np.float32)
    ts = np.asarray(inputs["timestamps"])
    pad = np.asarray(inputs["pad_mask"])
    uvqk = np.asarray(inputs["uvqk"], dtype=np.float32)
    o_w = np.asarray(inputs["o_w"], dtype=np.float32)
    o_b = np.asarray(inputs["o_b"], dtype=np.float32)
    ln_x_w = np.asarray(inputs["ln_x_w"], dtype=np.float32)
    ln_x_b = np.asarray(inputs["ln_x_b"], dtype=np.float32)
    ln_a_w = np.asarray(inputs["ln_a_w"], dtype=np.float32)
    ln_a_b = np.asarray(inputs["ln_a_b"], dtype=np.float32)
    ts_w = np.asarray(inputs["ts_w"], dtype=np.float32)
    pos_w = np.asarray(inputs["pos_w"], dtype=np.float32)
    assert not np.any(ln_x_b) and not np.any(ln_a_b), "nonzero LN bias unsupported"

    if "nc" not in _CACHE:
        _CACHE["nc"] = _build_nc()
        _CACHE["bt"] = _bucket_table()
        _CACHE["M"] = _build_M().reshape(128, 4 * 512)
    nc = _CACHE["nc"]
    buck, T, kmax = _CACHE["bt"]
    Mh = _CACHE["M"]

    uvqk_f = ln_x_w[:, None] * uvqk          # fold ln_x_w
    o_w_f = ln_a_w[:, None] * o_w            # fold ln_a_w
    idb = np.eye(128, dtype=np.float16)
    obr = o_b.reshape(1, D).astype(np.float16)

    ekey = (ts.tobytes(), ts_w.tobytes(), pos_w.tobytes())
    if _CACHE.get("ekey") == ekey:
        Es = _CACHE["Es"]
    else:
        Es = [_build_E(ts[b], ts_w, pos_w, buck, T, kmax).astype(np.float16)
              for b in range(B)]
        _CACHE["ekey"] = ekey
        _CACHE["Es"] = Es

    in_maps = []
    for c in range(8):
        b, hg = c // 2, c % 2
        ucols = uvqk_f[:, 256 * hg: 256 * hg + 256]
        vcols = uvqk_f[:, 512 + 256 * hg: 512 + 256 * hg + 256]
        qcols = uvqk_f[:, 1024 + 256 * hg: 1024 + 256 * hg + 256]
        kcols = uvqk_f[:, 1536 + 256 * hg: 1536 + 256 * hg + 256]
        wuv = np.concatenate([ucols, vcols, qcols, kcols], axis=1).astype(np.float16)
        wo_own = o_w_f[256 * hg: 256 * hg + 256].astype(np.float16)
        pf = (~pad[b]).astype(np.float32)
        padv = (pf / N).reshape(NT, 128).T.copy()
        padr = pf.reshape(NT, 128).T.copy()
        # x-residual and o_b folded in on the even core only (summed by the
        # pairwise AllReduce that writes the final output)
        even = float(hg == 0)
        padxr = (padr * even).astype(np.float32)
        ones1 = np.full((1, 128), even, dtype=np.float16)
        in_maps.append(dict(
            x=x[b], wuv=wuv, wo=wo_own, ob=obr, E=Es[b], M=Mh,
            padv=padv, padr=padr, padxr=padxr, idb=idb, ones1=ones1,
        ))

    from concourse.bass_utils import run_bass_kernel_spmd
    import os, time as _time
    _t0 = _time.time()
    try:
        res = run_bass_kernel_spmd(nc, in_maps, core_ids=list(range(8)),
                                   trace=bool(int(os.environ.get("KTRACE", "0"))))
    except ModuleNotFoundError:
        res = run_bass_kernel_spmd(nc, in_maps, core_ids=list(range(8)))
    _CACHE["last"] = res
    _CACHE["dev_wall"] = _time.time() - _t0
    out = np.stack([res.results[2 * b]["out"] for b in range(B)], axis=0)
    return out


# revision 78
# speedup vs baseline: 1.0006x; 1.0006x over previous
"""HSTU block kernel for 8 trn2 NeuronCores (v3).

Sharding: core c -> batch c//2, head-group c%2 (4 of 8 heads).
Per pair (2b, 2b+1): attention head-split; LN(attn) stats via tiny pairwise
AllReduces (two halves, overlapped with attention); output projection
computed as partial sums over each core's 256 attn dims and pairwise
AllReduce-added straight into the output tensor.

The rel-bias staircase is expanded on device by a prefix-sum scan (gpsimd)
over a host-staged impulse canvas E:
 - each 512-col chunk's start column holds the exact bias, so scan chunks
   are independent (init 0) and start at column 512*(jt//4);
 - on diagonal chunks the scan's second operand adds exact +-64 steps that
   make silu(qk + bias) vanish above the causal diagonal (no mask pass).

Engine budget per core: PE does projections/attention/output matmuls (fp16
operands, f32 psum), ACT does all silu/LN work with batched Sqrt (3 table
switches total), DVE does reductions/casts/psum evacuation, GpSimd does the
scan + collectives, DMA engines do the fp16 transposes (XBAR).
"""

import numpy as np
from contextlib import ExitStack

B, N, D = 4, 2048, 512
H, DV, DQ = 8, 64, 64
NT = N // 128          # 16 token tiles
EPS = 1e-5
MASK_OFF = 64.0        # silu(qk + bias - 64) == 0 above the causal diagonal

_CACHE = {}


# ---------------------------------------------------------------- host metadata
def _bucket_table():
    d_all = np.arange(0, 1000001, dtype=np.float32)
    buck = np.clip((np.log(np.maximum(d_all, 1.0)) / np.float32(0.301)).astype(np.int32), 0, 128)
    kmax = int(buck.max())
    T = np.searchsorted(buck, np.arange(1, kmax + 1), side="left")
    return buck, T, kmax


def _build_E(ts_b, ts_w, pos_w, buck, T, kmax):
    """Impulse canvas E [j, i]: cumsum along i from each 512-chunk start
    column reproduces bias^T (key-major)."""
    c = ts_b.astype(np.int64)
    r = np.concatenate([ts_b[1:], ts_b[-1:]]).astype(np.int64)
    tw = ts_w.astype(np.float32)
    delta = tw[1:kmax + 1] - tw[0:kmax]
    E = np.zeros((N, N), dtype=np.float32)
    Dp = (pos_w[:-1] - pos_w[1:]).astype(np.float32)
    jj = np.arange(N)
    ii = np.arange(1, N)
    # Toeplitz pos deltas: E[j, i>=1] += Dp[N-1+j-i]
    E[:, 1:] += Dp[(N - 1 + jj[:, None] - ii[None, :])]
    for k in range(kmax):
        lo = np.searchsorted(r, c - T[k], side="right")
        hi = np.searchsorted(r, c + T[k], side="left")
        valid = lo < hi
        l2, h2, jv = lo[valid], hi[valid], jj[valid]
        m = (l2 >= 1) & (l2 < N)
        np.add.at(E, (jv[m], l2[m]), -delta[k])
        m = (h2 >= 1) & (h2 < N)
        np.add.at(E, (jv[m], h2[m]), delta[k])
    # exact bias at every 512-chunk start column >= the row's group start
    for ci in range(4):
        s = 512 * ci
        rows = jj[(jj // 512) <= ci]
        d0 = np.abs(r[s] - c[rows])
        base = tw[buck[d0]] + pos_w[N - 1 + rows - s].astype(np.float32)
        E[rows, s] = base
    return E


def _build_M():
    """Mask-step canvas for diagonal chunks, [128, 4, 512] (indexed by
    jt%4): -MASK_OFF at local col 0 (rows below the chunk-start row),
    +MASK_OFF at the causal diagonal. Exact in fp16."""
    M = np.zeros((128, 4, 512), dtype=np.float32)
    for q in range(4):
        for p in range(128):
            dcol = 128 * q + p
            if dcol > 0:
                M[p, q, 0] -= MASK_OFF
                M[p, q, dcol] += MASK_OFF
    return M.astype(np.float16)


# ---------------------------------------------------------------- device kernel
def _build_nc(no_cc=False):
    import concourse.bass as bass
    import concourse.bacc as bacc
    import concourse.mybir as mybir
    import concourse.tile as tile

    f32 = mybir.dt.float32
    f16 = mybir.dt.float16
    AF = mybir.ActivationFunctionType
    ALU = mybir.AluOpType
    AX = mybir.AxisListType

    nc = bacc.Bacc(num_devices=8)

    x_in = nc.dram_tensor("x", [N, D], f32, kind="ExternalInput")
    wuv_in = nc.dram_tensor("wuv", [D, 1024], f16, kind="ExternalInput")
    wo_in = nc.dram_tensor("wo", [256, D], f16, kind="ExternalInput")
    ob_in = nc.dram_tensor("ob", [1, D], f16, kind="ExternalInput")
    E_in = nc.dram_tensor("E", [N, N], f16, kind="ExternalInput")
    padv_in = nc.dram_tensor("padv", [128, NT], f32, kind="ExternalInput")
    padr_in = nc.dram_tensor("padr", [128, NT], f32, kind="ExternalInput")
    idb_in = nc.dram_tensor("idb", [128, 128], f16, kind="ExternalInput")
    ones_in = nc.dram_tensor("ones1", [1, 128], f16, kind="ExternalInput")
    M_in = nc.dram_tensor("M", [128, 4 * 512], f16, kind="ExternalInput")
    padxr_in = nc.dram_tensor("padxr", [128, NT], f32, kind="ExternalInput")
    sin_t = [nc.dram_tensor(f"sin{h}", [128, 16], f32) for h in range(2)]
    sout_t = [nc.dram_tensor(f"sout{h}", [128, 16], f32) for h in range(2)]
    opart_t = nc.dram_tensor("opart", [N, D], f32)
    ored_t = nc.dram_tensor("ored", [N, D], f32)
    out_t = nc.dram_tensor("out", [N, D], f32, kind="ExternalOutput")

    pairs = [[0, 1], [2, 3], [4, 5], [6, 7]]

    with tile.TileContext(nc) as tc, ExitStack() as top:
        cpool = top.enter_context(tc.tile_pool(name="consts", bufs=1))
        idb = cpool.tile([128, 128], f16)
        ones1 = cpool.tile([1, 128], f16)
        obr = cpool.tile([1, D], f16)
        padv = cpool.tile([128, NT], f32)
        padr = cpool.tile([128, NT], f32)
        padxr = cpool.tile([128, NT], f32)
        epst = cpool.tile([128, 1], f32)
        nc.vector.memset(epst[:], EPS)
        nc.sync.dma_start(padxr[:], padxr_in[:, :])
        Mt = cpool.tile([128, 4, 512], f16)
        wq = [cpool.tile([128, 1024], f16, tag=f"wq{k}", name=f"wq{k}") for k in range(4)]
        wo = [cpool.tile([128, D], f16, tag=f"wo{k}", name=f"wo{k}") for k in range(2)]
        nc.sync.dma_start(Mt[:], M_in[:, :])
        nc.sync.dma_start(padv[:], padv_in[:, :])
        nc.sync.dma_start(padr[:], padr_in[:, :])
        nc.sync.dma_start(idb[:], idb_in[:, :])
        nc.sync.dma_start(ones1[:], ones_in[:, :])
        nc.sync.dma_start(obr[:], ob_in[:, :])
        for k in range(4):
            nc.sync.dma_start(wq[k][:], wuv_in[k * 128:(k + 1) * 128, :])
        for k in range(2):
            nc.sync.dma_start(wo[k][:], wo_in[k * 128:(k + 1) * 128, :])

        # resident activation storage
        rpool = top.enter_context(tc.tile_pool(name="resid", bufs=1))
        normT = rpool.tile([128, 4, N], f16)
        qT = [rpool.tile([128, N], f16, tag=f"qT{p}", name=f"qT{p}") for p in range(2)]
        kT = [rpool.tile([128, N], f16, tag=f"kT{p}", name=f"kT{p}") for p in range(2)]
        uvt = [rpool.tile([128, 512], f16, tag=f"uv{t}", name=f"uv{t}") for t in range(NT)]
        avt = [rpool.tile([128, 256], f16, tag=f"avt{t}", name=f"avt{t}") for t in range(NT)]
        bias = [rpool.tile([128, N - 512 * (jt // 4)], f16, tag=f"bias{jt}", name=f"bias{jt}")
                for jt in range(NT)]
        oT = rpool.tile([128, 2, N], f16)
        xf = [rpool.tile([128, D], f16, tag=f"xf{t}", name=f"xf{t}") for t in range(NT)]
        s1sum = rpool.tile([128, NT], f32)
        s1sq = rpool.tile([128, NT], f32)
        stS = rpool.tile([128, NT], f32)
        stQ = rpool.tile([128, NT], f32)
        mu1 = rpool.tile([128, NT], f32)
        rs1 = rpool.tile([128, NT], f32)
        nm1 = rpool.tile([128, NT], f32)
        rs5 = rpool.tile([128, NT], f32)
        nm5 = rpool.tile([128, NT], f32)

        # long-lived working pools
        xp = top.enter_context(tc.tile_pool(name="xly", bufs=1))
        scrp = top.enter_context(tc.tile_pool(name="scr", bufs=1))
        ep = top.enter_context(tc.tile_pool(name="escan", bufs=2))
        sp = top.enter_context(tc.tile_pool(name="stat", bufs=2))
        nrmp = top.enter_context(tc.tile_pool(name="nrm", bufs=2))
        oinp = top.enter_context(tc.tile_pool(name="oin", bufs=3))
        osbp = top.enter_context(tc.tile_pool(name="osb", bufs=2))
        s5p = top.enter_context(tc.tile_pool(name="stat5", bufs=2))
        wpool = top.enter_context(tc.tile_pool(name="wprime", bufs=1))
        scr4p = top.enter_context(tc.tile_pool(name="scr4", bufs=3))
        pbig = top.enter_context(tc.tile_pool(name="pbig", bufs=2, space="PSUM"))
        pqk = top.enter_context(tc.tile_pool(name="pqk", bufs=2, space="PSUM"))
        pav = top.enter_context(tc.tile_pool(name="pav", bufs=2, space="PSUM"))
        wp = [wpool.tile([128, 2, 512], f16, tag=f"wp{jt}", name=f"wp{jt}")
              for jt in range(NT)]

        # ----- thunk helpers ------------------------------------------------
        def emit(thunks):
            for th in thunks:
                th()

        def interleave(a, b):
            """Emit a and b round-robin, proportionally."""
            na, nb = len(a), len(b)
            if nb == 0:
                emit(a)
                return
            ia = ib = 0
            while ia < na or ib < nb:
                # keep a's progress ratio ahead of b's
                if ib >= nb or (ia < na and ia * nb <= ib * na):
                    a[ia]()
                    ia += 1
                else:
                    b[ib]()
                    ib += 1

        def scan_jt(jt, chunks=None):
            # walrus only lowers TensorTensorScanArith on DVE
            eng = nc.vector
            g = jt // 4
            cl = list(chunks) if chunks is not None else list(range(g, 4))
            c0 = cl[0]
            wdt = (cl[-1] + 1 - c0) * 512
            et = ep.tile([128, N], f16, tag="E")
            nc.sync.dma_start(et[:, :wdt],
                              E_in[jt * 128:(jt + 1) * 128,
                                   512 * c0:512 * c0 + wdt])
            for c in cl:
                lo = c * 512 - 512 * g          # col inside bias[jt]
                el = c * 512 - 512 * c0         # col inside the E tile
                if c == g:
                    eng.tensor_tensor_scan(bias[jt][:, lo:lo + 512],
                                           et[:, el:el + 512],
                                           Mt[:, jt % 4, :],
                                           0.0, ALU.add, ALU.add)
                else:
                    eng.tensor_tensor_scan(bias[jt][:, lo:lo + 512],
                                           et[:, el:el + 512], et[:, el:el + 512],
                                           0.0, ALU.add, ALU.bypass)

        xtiles = {}

        def stats_thunks(grp, ts0=None, ts1=None):
            ths = []
            for t in range(ts0 if ts0 is not None else grp * 8,
                           ts1 if ts1 is not None else grp * 8 + 8):
                def th(t=t):
                    xs = xp.tile([128, D], f32, tag=f"x{t % 9}")
                    xtiles[t] = xs
                    nc.sync.dma_start(xs[:], x_in[t * 128:(t + 1) * 128, :])
                    scr = scrp.tile([128, D], f16, tag="scr")
                    nc.scalar.activation(scr[:], xs[:], AF.Square,
                                         accum_out=s1sq[:, t:t + 1])
                    nc.vector.tensor_reduce(s1sum[:, t:t + 1], xs[:], AX.X, ALU.add)
                    if t < 4:
                        # earliest-needed bias chunk (attention c0) during
                        # the stats pass while DVE still has slack
                        scan_jt(t, chunks=[0])
                ths.append(th)
            return ths

        def batch1(grp, ts0=None, ts1=None):
            a = ts0 if ts0 is not None else grp * 8
            b = ts1 if ts1 is not None else grp * 8 + 8
            gs = slice(a, b)
            var1 = sp.tile([128, b - a], f32, tag="v1")
            nc.vector.tensor_scalar_mul(mu1[:, gs], s1sum[:, gs], 1.0 / D)
            nc.vector.scalar_tensor_tensor(var1[:], mu1[:, gs], -1.0, mu1[:, gs],
                                           ALU.mult, ALU.mult)
            nc.vector.scalar_tensor_tensor(var1[:], s1sq[:, gs], 1.0 / D, var1[:],
                                           ALU.mult, ALU.add)
            sd1 = sp.tile([128, b - a], f32, tag="sd1")
            nc.scalar.activation(sd1[:], var1[:], AF.Sqrt, bias=epst[:])
            nc.vector.reciprocal(rs1[:, gs], sd1[:])
            nc.vector.scalar_tensor_tensor(nm1[:, gs], mu1[:, gs], -1.0, rs1[:, gs],
                                           ALU.mult, ALU.mult)

        def apply_thunks(grp, ts0=None, ts1=None):
            ths = []
            for t in range(ts0 if ts0 is not None else grp * 8,
                           ts1 if ts1 is not None else grp * 8 + 8):
                def th(t=t):
                    nrm = nrmp.tile([128, D], f16, tag="nrm")
                    nc.scalar.activation(nrm[:], xtiles[t][:], AF.Identity,
                                         bias=nm1[:, t:t + 1], scale=rs1[:, t:t + 1])
                    nc.sync.dma_start_transpose(
                        normT[:, :, t * 128:(t + 1) * 128], nrm[:])
                    # fp16 cast of x with (pad-row x even-core) mask pre-folded
                    nc.vector.tensor_scalar_mul(xf[t][:], xtiles[t][:],
                                                padxr[:, t:t + 1])
                    if t >= 4:
                        scan_jt(t)
                    else:
                        scan_jt(t, chunks=[1, 2, 3])
                    ps = pbig.tile([128, 512], f32, tag="big")
                    for k in range(4):
                        nc.tensor.matmul(ps[:], normT[:, k, t * 128:(t + 1) * 128],
                                         wq[k][:, 0:512], start=(k == 0), stop=(k == 3))
                    nc.scalar.activation(uvt[t][:], ps[:], AF.Silu)
                    nc.vector.tensor_scalar_mul(uvt[t][:, 256:512],
                                                uvt[t][:, 256:512], padv[:, t:t + 1])
                    if t % 4 == 3:
                        c = t // 4
                        for p in range(2):
                            for col0 in (512, 768):  # q block, k block
                                ps = pbig.tile([128, 512], f32, tag="big")
                                for k in range(4):
                                    nc.tensor.matmul(
                                        ps[:],
                                        wq[k][:, col0 + p * 128:col0 + 128 + p * 128],
                                        normT[:, k, c * 512:(c + 1) * 512],
                                        start=(k == 0), stop=(k == 3))
                                dst = qT[p] if col0 == 512 else kT[p]
                                nc.scalar.activation(
                                    dst[:, c * 512:(c + 1) * 512], ps[:], AF.Silu)
                ths.append(th)
            return ths

        # ----- attention thunks --------------------------------------------
        def attn_thunks(P, c):
            ths = []
            for jt in range(4 * c + 4):
                def th(jt=jt, c=c, P=P):
                    g = jt // 4
                    off = 512 * g
                    lo = jt * 128 if c == g else c * 512
                    wdt = (c + 1) * 512 - lo
                    wlo = lo - c * 512
                    ps = pqk.tile([128, 2, 512], f32, tag="qk")
                    for hh in range(2):
                        nc.tensor.matmul(ps[:, hh, :wdt], idb[:],
                                         bias[jt][:, lo - off:lo - off + wdt],
                                         start=True, stop=False,
                                         skip_group_check=(hh == 1))
                        nc.tensor.matmul(
                            ps[:, hh, :wdt],
                            kT[P][64 * hh:64 * (hh + 1), jt * 128:(jt + 1) * 128],
                            qT[P][64 * hh:64 * (hh + 1), lo:lo + wdt],
                            start=False, stop=True, skip_group_check=True)
                    nc.scalar.activation(wp[jt][:, :, wlo:wlo + wdt],
                                         ps[:, :, :wdt], AF.Silu)
                ths.append(th)
            for it in range(4 * c, 4 * c + 4):
                def th(it=it, c=c, P=P):
                    pa = pav.tile([128, 128], f32, tag="av")
                    il = it * 128 - c * 512
                    # hh OUTER: interleaving two start=True groups in one
                    # bank loses the first group's accumulation (start
                    # clears the bank's has_written bits)
                    for hh in range(2):
                        for jt in range(it + 1):
                            nc.tensor.matmul(
                                pa[:, 64 * hh:64 * (hh + 1)],
                                wp[jt][:, hh, il:il + 128],
                                uvt[jt][:, 256 + 64 * (2 * P + hh):
                                         320 + 64 * (2 * P + hh)],
                                start=(jt == 0), stop=(jt == it),
                                skip_group_check=(hh == 1))
                    nc.vector.tensor_copy(avt[it][:, 128 * P:128 * (P + 1)], pa[:])
                    if P == 1:
                        # both pairs landed: per-token LN stats over all 256
                        # local attn dims in one pass
                        sq4 = scr4p.tile([128, 256], f16, tag="s4")
                        nc.vector.tensor_tensor(sq4[:], avt[it][:], avt[it][:],
                                                ALU.mult)
                        nc.vector.tensor_reduce(stQ[:, it:it + 1], sq4[:],
                                                AX.X, ALU.add)
                        nc.vector.tensor_reduce(stS[:, it:it + 1], avt[it][:],
                                                AX.X, ALU.add)
                ths.append(th)
            return ths

        def ship_stats(h):
            its = slice(h * 8, h * 8 + 8)
            nc.sync.dma_start(sin_t[h][:, 0:8], stS[:, its])
            nc.sync.dma_start(sin_t[h][:, 8:16], stQ[:, its])
            if not no_cc:
                nc.gpsimd.collective_compute(
                    "AllReduce", mybir.AluOpType.add, replica_groups=pairs,
                    ins=[sin_t[h][:, :]], outs=[sout_t[h][:, :]])

        def stats5(h):
            src = sin_t[h] if no_cc else sout_t[h]
            ar = s5p.tile([128, 16], f32, tag="ar")
            nc.sync.dma_start(ar[:], src[:, 0:16])
            gs = slice(h * 8, h * 8 + 8)
            mu5 = s5p.tile([128, 8], f32, tag="mu5")
            nc.vector.tensor_scalar_mul(mu5[:], ar[:, 0:8], 1.0 / 512)
            vr5 = s5p.tile([128, 8], f32, tag="vr5")
            nc.vector.scalar_tensor_tensor(vr5[:], mu5[:], -1.0, mu5[:],
                                           ALU.mult, ALU.mult)
            nc.vector.scalar_tensor_tensor(vr5[:], ar[:, 8:16], 1.0 / 512, vr5[:],
                                           ALU.mult, ALU.add)
            sd5 = s5p.tile([128, 8], f32, tag="sd5")
            nc.scalar.activation(sd5[:], vr5[:], AF.Sqrt, bias=epst[:])
            nc.vector.reciprocal(rs5[:, gs], sd5[:])
            nc.vector.scalar_tensor_tensor(nm5[:, gs], mu5[:], -1.0, rs5[:, gs],
                                           ALU.mult, ALU.mult)

        def phase5_thunks(h):
            ths = []
            for t in range(h * 8, h * 8 + 8):
                def th(t=t):
                    ln = oinp.tile([128, 256], f16, tag="ln")
                    nc.scalar.activation(ln[:], avt[t][:], AF.Identity,
                                         bias=nm5[:, t:t + 1], scale=rs5[:, t:t + 1])
                    oin = oinp.tile([128, 256], f16, tag="oin")
                    nc.vector.tensor_tensor(oin[:], ln[:], uvt[t][:, 0:256], ALU.mult)
                    nc.sync.dma_start_transpose(
                        oT[:, :, t * 128:(t + 1) * 128], oin[:])
                    ps = pbig.tile([128, 512], f32, tag="big")
                    for k in range(2):
                        nc.tensor.matmul(ps[:], oT[:, k, t * 128:(t + 1) * 128],
                                         wo[k][:], start=(k == 0), stop=False)
                    nc.tensor.matmul(ps[:], ones1[:], obr[:],
                                     start=False, stop=True, skip_group_check=True)
                    osb5 = osbp.tile([128, D], f32, tag="osb5")
                    nc.vector.scalar_tensor_tensor(osb5[:], ps[:], padr[:, t:t + 1],
                                                   xf[t][:], ALU.mult, ALU.add)
                    nc.sync.dma_start(opart_t[t * 128:(t + 1) * 128, :], osb5[:])
                ths.append(th)
            return ths

        def ship_out(h):
            rows = slice(1024 * h, 1024 * (h + 1))
            if not no_cc:
                nc.gpsimd.collective_compute(
                    "AllReduce", mybir.AluOpType.add, replica_groups=pairs,
                    ins=[opart_t[rows, :]], outs=[ored_t[rows, :]])
                nc.sync.dma_start(out_t[rows, :], ored_t[rows, :])

        # ----- schedule -----------------------------------------------------
        emit(stats_thunks(0, 0, 4))
        batch1(0, 0, 4)
        interleave(apply_thunks(0, 0, 4), stats_thunks(0, 4, 8))
        batch1(0, 4, 8)
        interleave(attn_thunks(0, 0), apply_thunks(0, 4, 8) + stats_thunks(1, 8, 12))
        interleave(attn_thunks(1, 0), stats_thunks(1, 12, 16))
        batch1(1)
        interleave(attn_thunks(0, 1) + attn_thunks(1, 1), apply_thunks(1))
        ship_stats(0)
        emit(attn_thunks(0, 2) + attn_thunks(1, 2))
        stats5(0)
        interleave(attn_thunks(0, 3) + attn_thunks(1, 3), phase5_thunks(0))
        ship_stats(1)
        ship_out(0)
        stats5(1)
        emit(phase5_thunks(1))
        ship_out(1)
        if no_cc:
            # sim-only fallback so out_t is produced without collectives
            nc.sync.dma_start(out_t[:, :], opart_t[:, :])

    nc.compile()
    return nc


# ---------------------------------------------------------------- entry point
def kernel(**inputs):
    x = np.asarray(inputs["x"], dtype=np.float32)
    ts = np.asarray(inputs["timestamps"])
    pad = np.asarray(inputs["pad_mask"])
    uvqk = np.asarray(inputs["uvqk"], dtype=np.float32)
    o_w = np.asarray(inputs["o_w"], dtype=np.float32)
    o_b = np.asarray(inputs["o_b"], dtype=np.float32)
    ln_x_w = np.asarray(inputs["ln_x_w"], dtype=np.float32)
    ln_x_b = np.asarray(inputs["ln_x_b"], dtype=np.float32)
    ln_a_w = np.asarray(inputs["ln_a_w"], dtype=np.float32)
    ln_a_b = np.asarray(inputs["ln_a_b"], dtype=np.float32)
    ts_w = np.asarray(inputs["ts_w"], dtype=np.float32)
    pos_w = np.asarray(inputs["pos_w"], dtype=np.float32)
    assert not np.any(ln_x_b) and not np.any(ln_a_b), "nonzero LN bias unsupported"

    if "nc" not in _CACHE:
        _CACHE["nc"] = _build_nc()
        _CACHE["bt"] = _bucket_table()
        _CACHE["M"] = _build_M().reshape(128, 4 * 512)
    nc = _CACHE["nc"]
    buck, T, kmax = _CACHE["bt"]
    Mh = _CACHE["M"]

    uvqk_f = ln_x_w[:, None] * uvqk          # fold ln_x_w
    o_w_f = ln_a_w[:, None] * o_w            # fold ln_a_w
    idb = np.eye(128, dtype=np.float16)
    obr = o_b.reshape(1, D).astype(np.float16)

    ekey = (ts.tobytes(), ts_w.tobytes(), pos_w.tobytes())
    if _CACHE.get("ekey") == ekey:
        Es = _CACHE["Es"]
    else:
        Es = [_build_E(ts[b], ts_w, pos_w, buck, T, kmax).astype(np.float16)
              for b in range(B)]
        _CACHE["ekey"] = ekey
        _CACHE["Es"] = Es

    in_maps = []
    for c in range(8):
        b, hg = c // 2, c % 2
        ucols = uvqk_f[:, 256 * hg: 256 * hg + 256]
        vcols = uvqk_f[:, 512 + 256 * hg: 512 + 256 * hg + 256]
        qcols = uvqk_f[:, 1024 + 256 * hg: 1024 + 256 * hg + 256]
        kcols = uvqk_f[:, 1536 + 256 * hg: 1536 + 256 * hg + 256]
        wuv = np.concatenate([ucols, vcols, qcols, kcols], axis=1).astype(np.float16)
        wo_own = o_w_f[256 * hg: 256 * hg + 256].astype(np.float16)
        pf = (~pad[b]).astype(np.float32)
        padv = (pf / N).reshape(NT, 128).T.copy()
        padr = pf.reshape(NT, 128).T.copy()
        # x-residual and o_b folded in on the even core only (summed by the
        # pairwise AllReduce that writes the final output)
        even = float(hg == 0)
        padxr = (padr * even).astype(np.float32)
        ones1 = np.full((1, 128), even, dtype=np.float16)
        in_maps.append(dict(
            x=x[b], wuv=wuv, wo=wo_own, ob=obr, E=Es[b], M=Mh,
            padv=padv, padr=padr, padxr=padxr, idb=idb, ones1=ones1,
        ))

    from concourse.bass_utils import run_bass_kernel_spmd
    import os, time as _time
    _t0 = _time.time()
    try:
        res = run_bass_kernel_spmd(nc, in_maps, core_ids=list(range(8)),
                                   trace=bool(int(os.environ.get("KTRACE", "0"))))
    except ModuleNotFoundError:
        res = run_bass_kernel_spmd(nc, in_maps, core_ids=list(range(8)))
    _CACHE["last"] = res
    _CACHE["dev_wall"] = _time.time() - _t0
    out = np.stack([res.results[2 * b]["out"] for b in range(B)], axis=0)
    return out
